# revision 5
# baseline (speedup 1.0000x reference)
"""Trainium2 Bass kernel for nn_MixedHead_17480516895445.

Math collapse (verified vs the jax reference to fp32 rounding):
  - q = emb[t] is broadcast over the query index, so layer-0 attention
    output is identical for all N=16 target rows; later layers are
    per-row, so the stack collapses to one row per token and the final
    mean over rows is the identity.
  - Layer-0 logits are linear in the features with only 16 distinct
    query vectors: dots = f @ B_class, B_j = SCALE*fold(Wf_k, emb[j]).
    The k-half of the feature matmul is never computed on device.
  - Self-attn layers 1/2 see identical rows: log_softmax of a constant
    row is -log N, so they reduce to x = (-N logN)(y@Wv)@Wo + bo + y.

Sharding: data-parallel, 512 tokens/core on 8 cores, weights replicated.
Device layout is column-major (feature dim on partitions, tokens free).
All matmuls in float32r: fp32 storage, 1 cycle/row on PE at N>=256.
"""

import sys

sys.path.insert(0, "/opt/trn_rl_repo")

import contextlib

import numpy as np

import concourse.bacc as bacc
import concourse.mybir as mybir
import concourse.tile as tile
from concourse.bass_utils import run_bass_kernel_spmd

B, T, INP, E, H, N = 16, 256, 1536, 512, 8, 16
D = E // H
SCALE = D ** -0.5
EPS = 1e-5
BT = B * T
NCORES = 8
TOK = BT // NCORES
P = 128
NKI = INP // P
NPE = E // P
NP2 = 2 * E // P
F32 = mybir.dt.float32
F32R = mybir.dt.float32r
AF = mybir.ActivationFunctionType
OP = mybir.AluOpType

_CACHE = {}
_BM = [0.0]
_LNB_ZERO = [True]


def _build(repeat: int = 1):
    nc = bacc.Bacc(None, target_bir_lowering=False)

    d = {}

    def din(name, shape, dt=F32R):
        d[name] = nc.dram_tensor(name, list(shape), dt, kind="ExternalInput")
        return d[name]

    fT = din("fT", (INP, TOK))
    Bcat = din("Bcat", (INP, N * P))
    Wfv = din("Wfv", (INP, E * N))
    onehot = din("onehot", (1, N * TOK))
    dbias = din("dbias", (P, N), F32)
    bfv = din("bfv", (P, 4 * N), F32)
    Rsel = din("Rsel", (N * NPE * P, P))
    G1 = din("G1", (P, H))
    G2 = din("G2", (H, P))
    I128 = din("I128", (P, P))
    ones_col = din("ones_col", (P, 1))
    ones_row = din("ones_row", (1, P))
    Wo0 = din("Wo0", (E, E))
    bo0 = din("bo0", (P, NPE), F32)
    lnw = din("lnw", (P, 5 * NPE), F32)
    lnb = din("lnb", (P, 5 * NPE), F32)
    W1s = din("W1s", (3 * E, 2 * E))
    b1s = din("b1s", (P, 3 * NP2), F32)
    W2s = din("W2s", (3 * 2 * E, E))
    b2s = din("b2s", (P, 3 * NPE), F32)
    Wvs = din("Wvs", (2 * E, E))
    Wos = din("Wos", (2 * E, E))
    bos = din("bos", (P, 2 * NPE), F32)
    Wm = din("Wm", (E, 1))
    epsc = din("epsc", (1, 1), F32)
    out = nc.dram_tensor("out", [1, TOK], F32, kind="ExternalOutput")

    with tile.TileContext(nc) as tc, contextlib.ExitStack() as ctx:
        sb_c = ctx.enter_context(tc.tile_pool(name="sb_c", bufs=1))
        sb_w = ctx.enter_context(tc.tile_pool(name="sb_w", bufs=1))
        sb_a = ctx.enter_context(tc.tile_pool(name="sb_a", bufs=1))
        ps_mm = ctx.enter_context(tc.tile_pool(name="ps_mm", bufs=1, space="PSUM"))
        ps_br = ctx.enter_context(tc.tile_pool(name="ps_br", bufs=1, space="PSUM"))
        ps_o = ctx.enter_context(tc.tile_pool(name="ps_o", bufs=1, space="PSUM"))

        uid = [0]

        def nm(pre):
            uid[0] += 1
            return f"{pre}_{uid[0]}"

        def wtile(tag, bufs):
            return sb_w.tile([P, P], F32R, name=nm(tag), tag=tag, bufs=bufs)

        def atile(tag, bufs, shape=(P, TOK), dt=F32R):
            return sb_a.tile(list(shape), dt, name=nm(tag), tag=tag, bufs=bufs)

        def pmm():
            return ps_mm.tile([P, TOK], F32, name=nm("mm"), tag="mm", bufs=2)

        def pbr(shape=(P, TOK)):
            return ps_br.tile(list(shape), F32, name=nm("br"), tag="br", bufs=2)

        # ---- resident constants -----------------------------------------
        ft = [sb_c.tile([P, TOK], F32R, name=f"ft{k}") for k in range(NKI)]
        for k in range(NKI):
            nc.sync.dma_start(ft[k][:], fT[k * P:(k + 1) * P, :])
        g1 = sb_c.tile([P, H], F32R)
        nc.sync.dma_start(g1[:], G1[:])
        g2 = sb_c.tile([H, P], F32R)
        nc.sync.dma_start(g2[:], G2[:])
        ident = sb_c.tile([P, P], F32R)
        nc.sync.dma_start(ident[:], I128[:])
        onec = sb_c.tile([P, 1], F32R)
        nc.sync.dma_start(onec[:], ones_col[:])
        oner = sb_c.tile([1, P], F32R)
        nc.sync.dma_start(oner[:], ones_row[:])
        dbias_sb = sb_c.tile([P, N], F32)
        nc.sync.dma_start(dbias_sb[:], dbias[:])
        bfv_sb = sb_c.tile([P, 4 * N], F32)
        nc.sync.dma_start(bfv_sb[:], bfv[:])
        bo0_sb = sb_c.tile([P, NPE], F32)
        nc.sync.dma_start(bo0_sb[:], bo0[:])
        lnw_sb = sb_c.tile([P, 5 * NPE], F32)
        nc.sync.dma_start(lnw_sb[:], lnw[:])
        lnb_sb = sb_c.tile([P, 5 * NPE], F32)
        nc.sync.dma_start(lnb_sb[:], lnb[:])
        b1_sb = sb_c.tile([P, 3 * NP2], F32)
        nc.sync.dma_start(b1_sb[:], b1s[:])
        b2_sb = sb_c.tile([P, 3 * NPE], F32)
        nc.sync.dma_start(b2_sb[:], b2s[:])
        bos_sb = sb_c.tile([P, 2 * NPE], F32)
        nc.sync.dma_start(bos_sb[:], bos[:])
        oh_sb = sb_c.tile([1, N * TOK], F32R)
        nc.sync.dma_start(oh_sb[:], onehot[:])
        eps_sb = sb_c.tile([1, 1], F32)
        nc.sync.dma_start(eps_sb[:], epsc[:])
        wmt = [sb_c.tile([P, 1], F32R, name=f"wmt{p4}") for p4 in range(NPE)]
        for p4 in range(NPE):
            nc.sync.dma_start(wmt[p4][:], Wm[p4 * P:(p4 + 1) * P, :])

        def body():
            # ===== phase A: per-class logits + one-hot select ============
            dots = atile("keep", 8)
            for j in range(N):
                pd = pmm()
                for k in range(NKI):
                    bw = wtile("bw", 14)
                    nc.sync.dma_start(
                        bw[:], Bcat[k * P:(k + 1) * P, j * P:(j + 1) * P])
                    nc.tensor.matmul(pd[:], bw[:], ft[k][:],
                                     start=(k == 0), stop=(k == NKI - 1))
                pm = pbr()
                nc.tensor.matmul(pm[:], oner[:], oh_sb[:, j * TOK:(j + 1) * TOK],
                                 start=True, stop=True)
                msk = atile("msk", 3)
                nc.vector.tensor_copy(msk[:], pm[:])
                tmp = atile("tmp", 3)
                nc.vector.scalar_tensor_tensor(
                    out=tmp[:], in0=pd[:], scalar=dbias_sb[:, j:j + 1],
                    in1=msk[:], op0=OP.add, op1=OP.mult)
                if j == 0:
                    nc.vector.tensor_copy(dots[:], tmp[:])
                else:
                    nc.vector.tensor_tensor(out=dots[:], in0=dots[:],
                                            in1=tmp[:], op=OP.add)

            # ===== phase A2: log-softmax over m (column-major) ===========
            expd = atile("tmp", 3)
            nc.scalar.activation(expd[:], dots[:], AF.Exp)
            ps8 = pbr((H, TOK))
            nc.tensor.matmul(ps8[:], g1[:], expd[:], start=True, stop=True)
            ls8 = atile("row", 6, (H, TOK))
            nc.scalar.activation(ls8[:], ps8[:], AF.Ln)
            plr = pbr()
            nc.tensor.matmul(plr[:], g2[:], ls8[:], start=True, stop=True)
            a_sb = atile("keep", 8)
            nc.vector.tensor_tensor(out=a_sb[:], in0=dots[:], in1=plr[:],
                                    op=OP.subtract)

            # ===== phase B: v matmul + weighted sum into o ===============
            o_ps = [ps_o.tile([P, TOK], F32, name=nm("ops"), tag=f"o{p4}",
                              bufs=1) for p4 in range(NPE)]
            for m in range(N):
                for p4 in range(NPE):
                    col0 = m * E + p4 * P
                    pv = pmm()
                    for k in range(NKI):
                        vw = wtile("vw", 16)
                        nc.sync.dma_start(
                            vw[:], Wfv[k * P:(k + 1) * P, col0:col0 + P])
                        nc.tensor.matmul(pv[:], vw[:], ft[k][:],
                                         start=(k == 0), stop=(k == NKI - 1))
                    vt = atile("vt", 5)
                    nc.scalar.activation(
                        vt[:], pv[:], AF.Identity,
                        bias=bfv_sb[:, m * NPE + p4:m * NPE + p4 + 1])
                    rw = wtile("rw", 4)
                    r0 = (m * NPE + p4) * P
                    nc.sync.dma_start(rw[:], Rsel[r0:r0 + P, :])
                    pa = pbr()
                    nc.tensor.matmul(pa[:], rw[:], a_sb[:], start=True,
                                     stop=True)
                    prod = atile("prod", 4)
                    nc.vector.tensor_tensor(out=prod[:], in0=pa[:], in1=vt[:],
                                            op=OP.mult)
                    nc.tensor.matmul(o_ps[p4][:], ident[:], prod[:],
                                     start=(m == 0), stop=(m == N - 1))
            o_sb = [atile("keep", 8) for _ in range(NPE)]
            for p4 in range(NPE):
                nc.vector.tensor_copy(o_sb[p4][:], o_ps[p4][:])

            # ===== E-stack helpers =======================================
            def matmul_block(w_dram, row0, in_tiles, n_in, n_out, evict):
                outs = []
                for p4 in range(n_out):
                    pp = pmm()
                    for k in range(n_in):
                        wt = wtile("wt", 16)
                        nc.sync.dma_start(
                            wt[:], w_dram[row0 + k * P:row0 + (k + 1) * P,
                                          p4 * P:(p4 + 1) * P])
                        nc.tensor.matmul(pp[:], wt[:], in_tiles[k][:],
                                         start=(k == 0), stop=(k == n_in - 1))
                    outs.append(evict(pp, p4))
                return outs

            def layernorm(x_tiles, ln_idx):
                ps1 = pbr((1, TOK))
                ps2 = pbr((1, TOK))
                xsq = []
                for p4 in range(NPE):
                    sq = atile("tmp", 3)
                    nc.scalar.activation(sq[:], x_tiles[p4][:], AF.Square)
                    xsq.append(sq)
                for p4 in range(NPE):
                    nc.tensor.matmul(ps1[:], onec[:], x_tiles[p4][:],
                                     start=(p4 == 0), stop=(p4 == NPE - 1))
                for p4 in range(NPE):
                    nc.tensor.matmul(ps2[:], onec[:], xsq[p4][:],
                                     start=(p4 == 0), stop=(p4 == NPE - 1))
                mean = atile("row", 6, (1, TOK))
                nc.scalar.activation(mean[:], ps1[:], AF.Copy, scale=1.0 / E)
                msq = atile("row", 6, (1, TOK), F32)
                nc.scalar.activation(msq[:], mean[:], AF.Square)
                var = atile("row", 6, (1, TOK), F32)
                nc.vector.scalar_tensor_tensor(
                    out=var[:], in0=ps2[:], scalar=1.0 / E, in1=msq[:],
                    op0=OP.mult, op1=OP.subtract)
                std = atile("row", 6, (1, TOK), F32)
                nc.scalar.activation(std[:], var[:], AF.Sqrt, bias=eps_sb[:])
                rstd = atile("row", 6, (1, TOK))
                with nc.allow_low_precision(reason="rstd broadcast via f32r matmul"):
                    nc.vector.reciprocal(rstd[:], std[:])
                pmr = pbr()
                nc.tensor.matmul(pmr[:], oner[:], mean[:], start=True,
                                 stop=True)
                prr = pbr()
                nc.tensor.matmul(prr[:], oner[:], rstd[:], start=True,
                                 stop=True)
                ys = []
                for p4 in range(NPE):
                    xc = atile("tmp", 3)
                    nc.vector.tensor_tensor(out=xc[:], in0=x_tiles[p4][:],
                                            in1=pmr[:], op=OP.subtract)
                    y = atile("act", 22)
                    g_col = lnw_sb[:, ln_idx * NPE + p4:ln_idx * NPE + p4 + 1]
                    nc.vector.scalar_tensor_tensor(
                        out=y[:], in0=xc[:], scalar=g_col, in1=prr[:],
                        op0=OP.mult, op1=OP.mult)
                    if not _LNB_ZERO[0]:
                        b_col = lnb_sb[:, ln_idx * NPE + p4:
                                       ln_idx * NPE + p4 + 1]
                        y2 = atile("act", 22)
                        nc.scalar.activation(y2[:], y[:], AF.Identity,
                                             bias=b_col)
                        y = y2
                    ys.append(y)
                return ys

            ln_idx = [0]

            def ev_bias(bias_sb, off):
                def ev(pp, p4):
                    xo = atile("act", 22)
                    nc.scalar.activation(xo[:], pp[:], AF.Identity,
                                         bias=bias_sb[:, off + p4:off + p4 + 1])
                    return xo
                return ev

            x = matmul_block(Wo0, 0, o_sb, NPE, NPE, ev_bias(bo0_sb, 0))

            def ff_layer(x, ffi):
                y = layernorm(x, ln_idx[0])
                ln_idx[0] += 1

                def ev_gelu(pp, p4):
                    h = atile("act", 22)
                    nc.scalar.activation(
                        h[:], pp[:], AF.Gelu,
                        bias=b1_sb[:, ffi * NP2 + p4:ffi * NP2 + p4 + 1])
                    return h

                h1 = matmul_block(W1s, ffi * E, y, NPE, NP2, ev_gelu)

                def ev_res(pp, p4):
                    xo = atile("act", 22)
                    nc.vector.scalar_tensor_tensor(
                        out=xo[:], in0=pp[:],
                        scalar=b2_sb[:, ffi * NPE + p4:ffi * NPE + p4 + 1],
                        op0=OP.add, in1=y[p4][:], op1=OP.add)
                    return xo

                return matmul_block(W2s, ffi * 2 * E, h1, NP2, NPE, ev_res)

            def att_layer(x, ai):
                y = layernorm(x, ln_idx[0])
                ln_idx[0] += 1

                def ev_plain(pp, p4):
                    vo = atile("act", 22)
                    nc.vector.tensor_copy(vo[:], pp[:])
                    return vo

                vv = matmul_block(Wvs, ai * E, y, NPE, NPE, ev_plain)

                def ev_res(pp, p4):
                    xo = atile("act", 22)
                    nc.vector.scalar_tensor_tensor(
                        out=xo[:], in0=pp[:],
                        scalar=bos_sb[:, ai * NPE + p4:ai * NPE + p4 + 1],
                        op0=OP.add, in1=y[p4][:], op1=OP.add)
                    return xo

                return matmul_block(Wos, ai * E, vv, NPE, NPE, ev_res)

            x = ff_layer(x, 0)
            x = att_layer(x, 0)
            x = ff_layer(x, 1)
            x = att_layer(x, 1)
            x = ff_layer(x, 2)

            pf = pbr((1, TOK))
            for p4 in range(NPE):
                nc.tensor.matmul(pf[:], wmt[p4][:], x[p4][:],
                                 start=(p4 == 0), stop=(p4 == NPE - 1))
            fo = atile("row", 6, (1, TOK), F32)
            nc.scalar.activation(fo[:], pf[:], AF.Copy, bias=_BM[0])
            nc.sync.dma_start(out[:], fo[:])

        for _ in range(repeat):
            body()

    nc.compile()
    return nc


def _prep(inputs):
    f = np.ascontiguousarray(
        np.asarray(inputs["features"], np.float32).reshape(BT, INP))
    t = np.asarray(inputs["targets_array"]).reshape(BT).astype(np.int64)
    emb = np.asarray(inputs["emb"], np.float32)
    Wf = np.asarray(inputs["Wf"], np.float32)
    bf = np.asarray(inputs["bf"], np.float32)

    Wf_k = Wf[:, :E * N].reshape(INP, N, H, D)
    bf_k = bf[:E * N].reshape(N, H, D)
    Wf_v = np.ascontiguousarray(Wf[:, E * N:])
    bf_v = bf[E * N:]
    embh = emb.reshape(N, H, D)

    # Bcat[c, j*128 + (h*16+m)] = SCALE * sum_d Wf_k[c,m,h,d]*emb[j,h,d]
    Bc = SCALE * np.einsum("cmhd,jhd->jhmc", Wf_k, embh)     # (j,h,m,c)
    Bcat = np.ascontiguousarray(
        Bc.reshape(N, P, INP).transpose(2, 0, 1).reshape(INP, N * P))
    db = SCALE * np.einsum("mhd,jhd->jhm", bf_k, embh)       # (j,h,m)
    dbias = np.ascontiguousarray(db.reshape(N, P).T)          # (128, N)
    bfv = np.ascontiguousarray(bf_v.reshape(N * NPE, P).T)    # (128, 64)

    hh = np.arange(P) // N
    G1 = np.zeros((P, H), np.float32)
    G1[np.arange(P), hh] = 1.0
    G2 = np.zeros((H, P), np.float32)
    G2[hh, np.arange(P)] = 1.0
    Rsel = np.zeros((N, NPE, P, P), np.float32)
    for m in range(N):
        for p4 in range(NPE):
            q = np.arange(P)
            h = 2 * p4 + q // D
            Rsel[m, p4, h * N + m, q] = 1.0
    Rsel = Rsel.reshape(N * NPE * P, P)

    lnw = np.zeros((P, 5 * NPE), np.float32)
    lnb = np.zeros((P, 5 * NPE), np.float32)
    ln_names = [("ff0_g", "ff0_b"), ("att1_g", "att1_b"), ("ff1_g", "ff1_b"),
                ("att2_g", "att2_b"), ("ff2_g", "ff2_b")]
    for i, (gn, bn) in enumerate(ln_names):
        lnw[:, i * NPE:(i + 1) * NPE] = np.asarray(
            inputs[gn], np.float32).reshape(NPE, P).T
        lnb[:, i * NPE:(i + 1) * NPE] = np.asarray(
            inputs[bn], np.float32).reshape(NPE, P).T
    _LNB_ZERO[0] = bool(np.abs(lnb).max() == 0.0)

    W1s = np.concatenate([np.asarray(inputs[f"ff{i}_W1"], np.float32)
                          for i in range(3)], 0)
    b1s = np.concatenate([np.asarray(inputs[f"ff{i}_b1"], np.float32)
                          .reshape(NP2, P).T for i in range(3)], 1)
    W2s = np.concatenate([np.asarray(inputs[f"ff{i}_W2"], np.float32)
                          for i in range(3)], 0)
    b2s = np.concatenate([np.asarray(inputs[f"ff{i}_b2"], np.float32)
                          .reshape(NPE, P).T for i in range(3)], 1)
    katt = -float(N) * np.log(float(N))
    Wvs = np.concatenate([katt * np.asarray(inputs[f"att{i}_Wqkv"],
                                            np.float32)[:, 2 * E:]
                          for i in (1, 2)], 0)
    Wos = np.concatenate([np.asarray(inputs[f"att{i}_Wo"], np.float32)
                          for i in (1, 2)], 0)
    bos = np.concatenate([np.asarray(inputs[f"att{i}_bo"], np.float32)
                          .reshape(NPE, P).T for i in (1, 2)], 1)
    bo0 = np.asarray(inputs["att0_bo"], np.float32).reshape(NPE, P).T
    Wo0 = np.asarray(inputs["att0_Wo"], np.float32)
    Wm = np.asarray(inputs["Wm"], np.float32).reshape(E, 1)
    _BM[0] = float(np.asarray(inputs["bm"], np.float32).reshape(-1)[0])

    shared = dict(Bcat=Bcat, Wfv=Wf_v, dbias=dbias, bfv=bfv, Rsel=Rsel,
                  G1=G1, G2=G2, I128=np.eye(P, dtype=np.float32),
                  ones_col=np.ones((P, 1), np.float32),
                  ones_row=np.ones((1, P), np.float32),
                  epsc=np.full((1, 1), EPS, np.float32),
                  Wo0=Wo0, bo0=bo0, lnw=lnw, lnb=lnb, W1s=W1s, b1s=b1s,
                  W2s=W2s, b2s=b2s, Wvs=Wvs, Wos=Wos, bos=bos, Wm=Wm)

    in_maps = []
    for c in range(NCORES):
        sl = slice(c * TOK, (c + 1) * TOK)
        fTc = np.ascontiguousarray(f[sl].T)
        oh = np.zeros((N, TOK), np.float32)
        oh[t[sl], np.arange(TOK)] = 1.0
        im = dict(shared)
        im.update(fT=fTc, onehot=oh.reshape(1, N * TOK))
        in_maps.append(im)
    return in_maps


def kernel(**inputs) -> np.ndarray:
    in_maps = _prep(inputs)
    if "nc" not in _CACHE:
        _CACHE["nc"] = _build(repeat=1)
    nc = _CACHE["nc"]
    res = run_bass_kernel_spmd(nc, in_maps, core_ids=list(range(NCORES)))
    full = np.concatenate([r["out"].reshape(TOK) for r in res.results])
    return full.astype(np.float32).reshape(B, T, 1)
